# revision 37
# baseline (speedup 1.0000x reference)
"""BERT-style dense transformer kernel for 8 Trainium2 NeuronCores.

Data-parallel over batch (B=4096 -> 512/core). Per core:
  embed (per-column [V,H] matmul) -> 4 transformer layers -> per-column head
  + log_softmax. Token-major master layout [128 tokens, H]; feature-major
  side tensors via PE transposes where matmuls need them as lhsT.

Attention is transpose-free and mask-free on the vector engines:
 - scores are computed directly in [k, q] orientation (lhsT=kT, rhs=qT);
 - the attention mask is folded into the score matmul as 10 augmented
   contraction rows (rank-10 additive mask): a -240 everywhere term, +15.5^2
   block-diagonal same-sample terms, and a -15.5^2 * (1-u_k) * u_q term that
   excludes masked keys for unmasked queries. exp(scale*(s-240)) underflows
   to ~1e-13 for every invalid pair; the +0.25 residue on valid pairs is a
   per-row constant factor that cancels in the softmax normalization.
   Masked queries (whose q vector is exactly 0 after LN of the constant-15
   fill) get uniform weights over their sample, matching the reference.
 - exp has no max-subtraction (valid scores are ~N(0, 0.2^2));
 - normalization: ones-matmul rowsum -> DVE reciprocal -> gpsimd partition
   broadcast -> one wide multiply.
Each layer runs in two passes (attention pass, FF pass, h2T spilled via
DRAM); pass A is software-pipelined (block n's o/Wo/LN2 tail is emitted
after block n+1's LN1/QKV/score head) so the in-order PE queue always has
independent matmuls between the cross-engine softmax chains. LN rstd uses
exp(-0.5*ln(var+eps)) and the act-table pass is overridden to force the
combined ln+exp table, so each pass needs a single activation table.
Matmul inputs bf16 (fp32 PSUM); residual/LN in fp32.
"""
import sys
sys.path.insert(0, '/opt/trn_rl_repo')
import numpy as np
import ml_dtypes

import concourse.bass as bass
import concourse.bacc as bacc
import concourse.tile as tile
from concourse import mybir
from concourse.bass_utils import run_bass_kernel_spmd
from concourse.masks import make_identity

F32, BF16, FP8 = mybir.dt.float32, mybir.dt.bfloat16, mybir.dt.float8e4
AF = mybir.ActivationFunctionType
ALU = mybir.AluOpType
AX = mybir.AxisListType
BF16NP = ml_dtypes.bfloat16
FP8NP = ml_dtypes.float8_e4m3fn
DR = mybir.MatmulPerfMode.DoubleRow

# Problem constants
B, C, V, H, NH, L = 4096, 16, 1000, 512, 8, 4
DK = H // NH          # 64
FF = 4 * H            # 2048
NCORES = 8
BS = B // NCORES      # 512 batch/core
T = BS * C            # 8192 tokens/core
P = 128
NT = T // P           # 64 token tiles
HC = H // P           # 4 feature chunks
FC = FF // P          # 16 ff chunks
SCALE = 1.0 / np.sqrt(DK)  # 0.125
GA = 15.5             # aug-row gain; GA^2 ~= 240 drives invalid pairs to -30
NAUG = 10
# V (=1000) contraction chunks for embed: 7 full 128-chunks + tail 104
VCH = [(i * 128, 128) for i in range(7)] + [(896, 104)]

TBT = 4               # token tiles per block
TB = TBT * P          # 512 tokens per block
NB = T // TB          # 16 blocks

_CACHED = {}
DEBUG = False


class _Bacc(bacc.Bacc):
    """Bacc whose act-table pass cannot pick the exp-only / ln-only tables,
    forcing Exp and Ln onto the combined natural_log_exp_and_others set.
    Table ids stay honest (positions are preserved; only contents of the
    narrow tables are hidden from the chooser)."""

    def insert_act_table_loads(self):
        import concourse.hw_specs as hws
        has_activation = any(
            isinstance(i, mybir.InstActivation)
            for b in self.main_func.blocks
            for i in b.instructions
        )
        if not has_activation:
            return
        tabs = [
            (k, set() if k in ("natural_log", "exp_and_others",
                               "exp_and_friends") else v)
            for k, v in hws.get_activation_tables(self.m.arch).items()
        ]
        import bass_rust as _bass_rust
        _bass_rust.insert_act_table_loads(self, tabs)


def build_kernel():
    nc = _Bacc(None)

    xTin = nc.dram_tensor("xTin", [C, V, BS], BF16, kind="ExternalInput")
    embW = nc.dram_tensor("embW", [C, V, H], BF16, kind="ExternalInput")
    wq = nc.dram_tensor("wq", [L, H, H], FP8, kind="ExternalInput")
    wk = nc.dram_tensor("wk", [L, H, H], FP8, kind="ExternalInput")
    wv = nc.dram_tensor("wv", [L, H, H], FP8, kind="ExternalInput")
    wo = nc.dram_tensor("wo", [L, H, H], FP8, kind="ExternalInput")
    w1 = nc.dram_tensor("w1", [L, H, FF], FP8, kind="ExternalInput")
    w2 = nc.dram_tensor("w2", [L, FF, H], FP8, kind="ExternalInput")
    headW = nc.dram_tensor("headW", [C, H, V], BF16, kind="ExternalInput")
    uemb = nc.dram_tensor("uemb", [C, BS // P, P, 1], F32, kind="ExternalInput")
    w15emb = nc.dram_tensor("w15emb", [C, BS // P, P, 1], F32, kind="ExternalInput")
    augk = nc.dram_tensor("augk", [16, NT, P], BF16, kind="ExternalInput")
    augq = nc.dram_tensor("augq", [16, NT, P], BF16, kind="ExternalInput")
    out = nc.dram_tensor("out", [BS, C, V], F32, kind="ExternalOutput")
    dbg = nc.dram_tensor("dbg", [1 + L, T, H], F32, kind="ExternalOutput") if DEBUG else None

    xbuf = nc.dram_tensor("xbuf", [T, H], F32)
    h2Tbuf = nc.dram_tensor("h2Tbuf", [NB, P, HC, TB], FP8)
    x_c = xbuf.rearrange("(n c) h -> n c h", c=C)  # [BS, C, H] token rows by (b, c)

    with tile.TileContext(nc) as tc:
        # ---------------- constants ----------------
        const_cm = tc.tile_pool(name="const", bufs=1)
        const = const_cm.__enter__()
        ident = const.tile([P, P], BF16)
        make_identity(nc, ident[:])
        eps_t = const.tile([P, 1], F32)
        nc.vector.memset(eps_t[:], 1e-6)
        ones_t = const.tile([P, 1], BF16)
        nc.vector.memset(ones_t[:], 1.0)
        mvA = const.tile([P, NT, 2], F32)
        augk_s = const.tile([16, NT, P], BF16)
        augq_s = const.tile([16, NT, P], BF16)
        nc.sync.dma_start(out=augk_s[:], in_=augk[:])
        nc.sync.dma_start(out=augq_s[:], in_=augq[:])

        # ---------------- embed phase ----------------
        with tc.tile_pool(name="e_w", bufs=2) as e_w, \
             tc.tile_pool(name="e_x", bufs=3) as e_x, \
             tc.tile_pool(name="e_sc", bufs=3) as e_sc, \
             tc.tile_pool(name="e_ps", bufs=6, space="PSUM") as e_ps:
            for c in range(C):
                wt = e_w.tile([P, len(VCH), H], BF16, tag="wt")
                nc.sync.dma_start(
                    out=wt[:, :7, :],
                    in_=embW[c, :896, :].rearrange("(k p) h -> p k h", p=P))
                nc.sync.dma_start(out=wt[:104, 7, :], in_=embW[c, 896:, :])
                xt = e_x.tile([P, len(VCH), BS], BF16, tag="xt")
                nc.sync.dma_start(
                    out=xt[:, :7, :],
                    in_=xTin[c, :896, :].rearrange("(k p) b -> p k b", p=P))
                nc.sync.dma_start(out=xt[:104, 7, :], in_=xTin[c, 896:, :])
                for bt in range(BS // P):
                    bsl = slice(bt * P, (bt + 1) * P)
                    ut = e_sc.tile([P, 1], F32, tag="ut")
                    wt15 = e_sc.tile([P, 1], F32, tag="wt15")
                    nc.sync.dma_start(out=ut[:], in_=uemb[c, bt, :, :])
                    nc.sync.dma_start(out=wt15[:], in_=w15emb[c, bt, :, :])
                    eps = e_ps.tile([P, H], F32, tag="eps")
                    for k, (v0, vn) in enumerate(VCH):
                        nc.tensor.matmul(eps[:], lhsT=xt[:vn, k, bsl],
                                         rhs=wt[:vn, k, :],
                                         start=(k == 0), stop=(k == len(VCH) - 1))
                    x0 = e_x.tile([P, H], F32, tag="x0")
                    # x0 = e*u + 15*(1-u), in Act's scale/bias form
                    nc.scalar.activation(out=x0[:], in_=eps[:], func=AF.Identity,
                                         bias=wt15[:], scale=ut[:])
                    nc.sync.dma_start(out=x_c[bsl, c, :], in_=x0[:])
                    if DEBUG:
                        nc.sync.dma_start(
                            out=dbg.rearrange("d (n c) h -> d n c h", c=C)[0, bsl, c, :],
                            in_=x0[:])

        # ---------------- transformer layers ----------------
        for l in range(L):
            # ---- pass A: LN1, QKV, attention, Wo residual, LN2, h2T spill
            with tc.tile_pool(name="wpool", bufs=1) as wp, \
                 tc.tile_pool(name="xp", bufs=4) as xp, \
                 tc.tile_pool(name="hp", bufs=3) as hp, \
                 tc.tile_pool(name="qkp", bufs=3) as qkp, \
                 tc.tile_pool(name="ap", bufs=3) as ap_, \
                 tc.tile_pool(name="sp", bufs=4) as sp_, \
                 tc.tile_pool(name="ps_g", bufs=2, space="PSUM") as ps_g, \
                 tc.tile_pool(name="ps_sc", bufs=2, space="PSUM") as ps_sc, \
                 tc.tile_pool(name="ps_tp", bufs=2, space="PSUM") as ps_tp, \
                 tc.tile_pool(name="ps_o", bufs=2, space="PSUM") as ps_o:
                wq_s = wp.tile([P, HC, H], FP8)
                wk_s = wp.tile([P, HC, H], FP8)
                wv_s = wp.tile([P, HC, H], FP8)
                wo_s = wp.tile([P, HC, H], FP8)
                for wt_, src in ((wq_s, wq), (wk_s, wk), (wv_s, wv), (wo_s, wo)):
                    nc.sync.dma_start(
                        out=wt_[:],
                        in_=src[l].rearrange("(k p) n -> p k n", p=P))

                def emit_head1(blk):
                    t0 = blk * TBT
                    tok0 = blk * TB
                    xs = xp.tile([P, TBT, H], F32, tag="xs")
                    nc.sync.dma_start(
                        out=xs[:],
                        in_=xbuf[tok0:tok0 + TB, :].rearrange("(t p) h -> p t h", p=P))

                    # LN1 -> h (bf16); hT feature-major via PE transpose
                    h = hp.tile([P, TBT, H], BF16, tag="h")
                    hT = hp.tile([P, HC, TB], FP8, tag="hT")
                    _layernorm4(nc, sp_, xs, h, eps_t, "l1",
                                mvb=(None if l == 0 else mvA[:, t0:t0 + TBT, :]))
                    for i in range(TBT):
                        tp = ps_tp.tile([P, HC, P], BF16, tag="tp")
                        for kc in range(HC):
                            nc.tensor.transpose(tp[:, kc, :],
                                                in_=h[:, i, kc * P:(kc + 1) * P],
                                                identity=ident[:])
                        nc.scalar.activation(out=hT[:, :, i * P:(i + 1) * P],
                                             in_=tp[:], func=AF.Identity)

                    # qT, kT feature-major [P(dpart), HC, TB]
                    qT = qkp.tile([P, HC, TB], BF16, tag="qT")
                    kT = qkp.tile([P, HC, TB], BF16, tag="kT")
                    for dst, wmat in ((qT, wq_s), (kT, wk_s)):
                        for oc in range(HC):
                            pq = ps_g.tile([P, TB], F32, tag="g")
                            for j in range(2):
                                nc.tensor.matmul(pq[:],
                                                 lhsT=wmat[:, 2 * j:2 * j + 2,
                                                           oc * P:(oc + 1) * P],
                                                 rhs=hT[:, 2 * j:2 * j + 2, :],
                                                 start=(j == 0), stop=(j == 1),
                                                 perf_mode=DR)
                            nc.scalar.activation(out=dst[:, oc, :], in_=pq[:],
                                                 func=AF.Identity)

                    # v token-major [P(tok), TBT, H]
                    v_s = ap_.tile([P, TBT, H], BF16, tag="v_s")
                    for i in range(TBT):
                        pv = ps_g.tile([P, H], F32, tag="g")
                        for j in range(2):
                            nc.tensor.matmul(pv[:],
                                             lhsT=hT[:, 2 * j:2 * j + 2,
                                                     i * P:(i + 1) * P],
                                             rhs=wv_s[:, 2 * j:2 * j + 2, :],
                                             start=(j == 0), stop=(j == 1),
                                             perf_mode=DR)
                        nc.vector.tensor_copy(out=v_s[:, i, :], in_=pv[:])
                    return (blk, xs, v_s, qT, kT)

                def emit_head2(st1):
                    blk, xs, v_s, qT, kT = st1
                    t0 = blk * TBT
                    # attention: scores in [k, q] orientation, mask via aug rows
                    ens = []
                    for i in range(TBT):
                        en = ap_.tile([P, NH, P], BF16, tag=f"en{i}")
                        ens.append(en)
                        for g in range(2):
                            g4 = slice(g * 4, (g + 1) * 4)
                            psc = ps_sc.tile([P, 4, P], F32, tag="sc")
                            for j in range(4):
                                hh = g * 4 + j
                                dch, drow = (hh * DK) // P, (hh * DK) % P
                                nc.tensor.matmul(
                                    psc[:, j, :],
                                    lhsT=kT[drow:drow + DK, dch, i * P:(i + 1) * P],
                                    rhs=qT[drow:drow + DK, dch, i * P:(i + 1) * P],
                                    start=True, stop=False)
                                nc.tensor.matmul(
                                    psc[:, j, :],
                                    lhsT=augk_s[0:NAUG, t0 + i, :],
                                    rhs=augq_s[0:NAUG, t0 + i, :],
                                    start=False, stop=True)
                            nc.scalar.activation(out=en[:, g4, :], in_=psc[:],
                                                 func=AF.Exp, scale=SCALE)
                    for i in range(TBT):
                        en = ens[i]
                        for g in range(2):
                            g4 = slice(g * 4, (g + 1) * 4)
                            prs = ps_sc.tile([1, 4 * P], F32, tag="sc")
                            nc.tensor.matmul(prs[:], lhsT=ones_t[:], rhs=en[:, g4, :],
                                             start=True, stop=True)
                            rcp = sp_.tile([1, 4 * P], BF16, tag="rcp")
                            with nc.allow_low_precision(
                                    reason="softmax denominators tolerate bf16"):
                                nc.vector.reciprocal(out=rcp[:], in_=prs[:])
                            rb = sp_.tile([P, 4 * P], BF16, tag="rb")
                            nc.gpsimd.partition_broadcast(rb[:], rcp[0:1, :])
                            nc.vector.tensor_tensor(out=en[:, g4, :], in0=en[:, g4, :],
                                                    in1=rb[:], op=ALU.mult)
                    return (blk, xs, v_s, ens)

                def emit_tail(st):
                    blk, xs, v_s, ens = st
                    tok0 = blk * TB
                    for i in range(TBT):
                        en = ens[i]
                        # o feature-major [P(dpart), HC, P(q)]
                        po = ps_o.tile([P, HC, P], F32, tag="po")
                        for hh in range(NH):
                            dch, drow = (hh * DK) // P, (hh * DK) % P
                            nc.tensor.matmul(po[drow:drow + DK, dch, :],
                                             lhsT=v_s[:, i, hh * DK:(hh + 1) * DK],
                                             rhs=en[:, hh, :], start=True, stop=True)
                        oT = ap_.tile([P, HC, P], FP8, tag="oT")
                        nc.scalar.activation(out=oT[:], in_=po[:], func=AF.Identity)
                        # Wo + residual
                        pwo = ps_o.tile([P, H], F32, tag="po")
                        for j in range(2):
                            nc.tensor.matmul(pwo[:],
                                             lhsT=oT[:, 2 * j:2 * j + 2, :],
                                             rhs=wo_s[:, 2 * j:2 * j + 2, :],
                                             start=(j == 0), stop=(j == 1),
                                             perf_mode=DR)
                        nc.vector.tensor_tensor(out=xs[:, i, :], in0=pwo[:],
                                                in1=xs[:, i, :], op=ALU.add)

                    # post-attention x back to DRAM
                    nc.sync.dma_start(
                        out=xbuf[tok0:tok0 + TB, :].rearrange("(t p) h -> p t h", p=P),
                        in_=xs[:])

                    # LN2 -> h2; h2T spilled to DRAM for pass B
                    h2 = hp.tile([P, TBT, H], BF16, tag="h2")
                    h2T = hp.tile([P, HC, TB], FP8, tag="h2T")
                    _layernorm4(nc, sp_, xs, h2, eps_t, "l2")
                    for i in range(TBT):
                        tp = ps_tp.tile([P, HC, P], BF16, tag="tp")
                        for kc in range(HC):
                            nc.tensor.transpose(tp[:, kc, :],
                                                in_=h2[:, i, kc * P:(kc + 1) * P],
                                                identity=ident[:])
                        nc.scalar.activation(out=h2T[:, :, i * P:(i + 1) * P],
                                              in_=tp[:], func=AF.Identity)
                    nc.sync.dma_start(out=h2Tbuf[blk], in_=h2T[:])

                prev = None
                for blk in range(NB):
                    st1 = emit_head1(blk)
                    if prev is not None:
                        emit_tail(prev)
                    prev = emit_head2(st1)
                emit_tail(prev)

            # ---- pass B: W1 + GELU + W2 + residual
            with tc.tile_pool(name="wpB", bufs=1) as wpB, \
                 tc.tile_pool(name="xpB", bufs=4) as xpB, \
                 tc.tile_pool(name="hpB", bufs=2) as hpB, \
                 tc.tile_pool(name="gp", bufs=2) as gp, \
                 tc.tile_pool(name="ps_B", bufs=3, space="PSUM") as ps_B, \
                 tc.tile_pool(name="ps_B2", bufs=2, space="PSUM") as ps_B2:
                w1_s = wpB.tile([P, HC, FF], FP8)
                w2_s = wpB.tile([P, FC, H], FP8)
                for wt_, src in ((w1_s, w1), (w2_s, w2)):
                    nc.sync.dma_start(
                        out=wt_[:],
                        in_=src[l].rearrange("(k p) n -> p k n", p=P))
                def emit_w2(stB2):
                    blk, xs, gT = stB2
                    tok0 = blk * TB
                    for i in range(TBT):
                        pw2 = ps_B2.tile([P, H], F32, tag="pw")
                        for j in range(FC // 2):
                            nc.tensor.matmul(pw2[:],
                                             lhsT=gT[:, 2 * j:2 * j + 2,
                                                     i * P:(i + 1) * P],
                                             rhs=w2_s[:, 2 * j:2 * j + 2, :],
                                             start=(j == 0), stop=(j == FC // 2 - 1),
                                             perf_mode=DR)
                        xo = xpB.tile([P, H], F32, tag="xo")
                        nc.vector.tensor_tensor(out=xo[:], in0=pw2[:], in1=xs[:, i, :],
                                                op=ALU.add)
                        if l < L - 1:
                            stats = xpB.tile([P, 6], F32, tag="stB")
                            nc.vector.bn_stats(out=stats[:], in_=xo[:])
                            nc.vector.bn_aggr(out=mvA[:, blk * TBT + i, :],
                                              in_=stats[:])
                        nc.sync.dma_start(
                            out=xbuf[tok0 + i * P:tok0 + (i + 1) * P, :], in_=xo[:])
                        if DEBUG:
                            nc.sync.dma_start(
                                out=dbg[1 + l, tok0 + i * P:tok0 + (i + 1) * P, :],
                                in_=xo[:])

                prevB = None
                for blk in range(NB):
                    tok0 = blk * TB
                    xs = xpB.tile([P, TBT, H], F32, tag="xs")
                    nc.sync.dma_start(
                        out=xs[:],
                        in_=xbuf[tok0:tok0 + TB, :].rearrange("(t p) h -> p t h", p=P))
                    h2T = hpB.tile([P, HC, TB], FP8, tag="h2T")
                    nc.sync.dma_start(out=h2T[:], in_=h2Tbuf[blk])
                    gT = gp.tile([P, FC, TB], FP8, tag="gT")
                    prev_pg = None
                    for fp in range(FC // 2):
                        pg = ps_B.tile([P, 2, TB], F32, tag="pg")
                        for half in range(2):
                            fo = 2 * fp + half
                            for j in range(2):
                                nc.tensor.matmul(pg[:, half, :],
                                                 lhsT=w1_s[:, 2 * j:2 * j + 2,
                                                           fo * P:(fo + 1) * P],
                                                 rhs=h2T[:, 2 * j:2 * j + 2, :],
                                                 start=(j == 0), stop=(j == 1),
                                                 perf_mode=DR)
                        if prev_pg is not None:
                            pfp, ppg = prev_pg
                            nc.scalar.activation(out=gT[:, 2 * pfp:2 * pfp + 2, :],
                                                 in_=ppg[:], func=AF.Gelu_apprx_tanh)
                        prev_pg = (fp, pg)
                    pfp, ppg = prev_pg
                    nc.scalar.activation(out=gT[:, 2 * pfp:2 * pfp + 2, :],
                                         in_=ppg[:], func=AF.Gelu_apprx_tanh)
                    if prevB is not None:
                        emit_w2(prevB)
                    prevB = (blk, xs, gT)
                emit_w2(prevB)

        # ---------------- head phase ----------------
        with tc.tile_pool(name="h_w", bufs=2) as h_w, \
             tc.tile_pool(name="h_x", bufs=3) as h_x, \
             tc.tile_pool(name="h_s", bufs=3) as h_s, \
             tc.tile_pool(name="h_ps", bufs=2, space="PSUM") as h_ps, \
             tc.tile_pool(name="h_pt", bufs=2, space="PSUM") as h_pt:
            for c in range(C):
                hw = h_w.tile([P, HC, V], BF16, tag="hw")
                nc.sync.dma_start(out=hw[:],
                                  in_=headW[c].rearrange("(k p) v -> p k v", p=P))
                for bt in range(BS // P):
                    bsl = slice(bt * P, (bt + 1) * P)
                    xc = h_x.tile([P, H], F32, tag="xc")
                    nc.sync.dma_start(out=xc[:], in_=x_c[bsl, c, :])
                    xcb = h_x.tile([P, H], BF16, tag="xcb")
                    nc.vector.tensor_copy(out=xcb[:], in_=xc[:])
                    xcT = h_x.tile([P, HC, P], BF16, tag="xcT")
                    tp = h_pt.tile([P, HC, P], BF16, tag="tp2")
                    for kc in range(HC):
                        nc.tensor.transpose(tp[:, kc, :],
                                            in_=xcb[:, kc * P:(kc + 1) * P],
                                            identity=ident[:])
                    nc.vector.tensor_copy(out=xcT[:], in_=tp[:])
                    lg = h_s.tile([P, V], F32, tag="hlg")
                    pl = h_ps.tile([P, 2, 512], F32, tag="pl")
                    for ng in range(2):
                        nsl = slice(ng * 500, (ng + 1) * 500)
                        for ki in range(HC):
                            nc.tensor.matmul(pl[:, ng, :500], lhsT=xcT[:, ki, :],
                                             rhs=hw[:, ki, nsl],
                                             start=(ki == 0), stop=(ki == HC - 1))
                    nc.vector.tensor_copy(out=lg[:].rearrange("p (n v) -> p n v", n=2),
                                          in_=pl[:, :, :500])
                    # log_softmax over V (no max subtraction: logits are far
                    # from f32 exp overflow)
                    ex = h_s.tile([P, V], F32, tag="hex")
                    sm = h_s.tile([P, 1], F32, tag="hsm")
                    nc.scalar.activation(out=ex[:], in_=lg[:], func=AF.Exp,
                                         scale=1.0, accum_out=sm[:])
                    lnz = h_s.tile([P, 1], F32, tag="hlnz")
                    nc.scalar.activation(out=lnz[:], in_=sm[:], func=AF.Ln)
                    off = h_s.tile([P, 1], F32, tag="hoff")
                    nc.vector.tensor_scalar(out=off[:], in0=lnz[:], scalar1=-1.0,
                                            scalar2=None, op0=ALU.mult)
                    lo = h_s.tile([P, V], F32, tag="hlo")
                    nc.vector.tensor_scalar(out=lo[:], in0=lg[:], scalar1=off[:],
                                            scalar2=None, op0=ALU.add)
                    nc.sync.dma_start(out=out[bsl, c, :], in_=lo[:])

        const_cm.__exit__(None, None, None)

    nc.finalize()
    return nc


def _layernorm4(nc, pool, xs, h, eps_t, tag, mvb=None):
    """Batched LN over TBT tiles: h[:, i, :] = (x - mean_i) * rsqrt(var_i + eps).

    rstd is computed as exp(-0.5 * ln(var + eps)) so the Activation engine
    stays inside the combined exp/ln function table (no act-table reload).
    If mvb is given, the per-tile mean/var were precomputed (forwarded from
    the previous layer's FF pass) and the stats step is skipped.
    """
    if mvb is None:
        mvb = pool.tile([P, TBT, 2], F32, tag=f"mv_{tag}")
        for i in range(TBT):
            stats = pool.tile([P, 6], F32, tag=f"st_{tag}{i}")
            nc.vector.bn_stats(out=stats[:], in_=xs[:, i, :])
            nc.vector.bn_aggr(out=mvb[:, i, :], in_=stats[:])
    lnv = pool.tile([P, TBT], F32, tag=f"lv_{tag}")
    nc.scalar.activation(out=lnv[:], in_=mvb[:, :, 1], func=AF.Ln,
                         bias=eps_t[:], scale=1.0)
    rstd = pool.tile([P, TBT], F32, tag=f"rs_{tag}")
    nc.scalar.activation(out=rstd[:], in_=lnv[:], func=AF.Exp, scale=-0.5)
    nmb = pool.tile([P, TBT], F32, tag=f"nm_{tag}")
    nc.vector.tensor_tensor(out=nmb[:], in0=mvb[:, :, 0], in1=rstd[:], op=ALU.mult)
    nc.vector.tensor_scalar(out=nmb[:], in0=nmb[:], scalar1=-1.0, scalar2=None,
                            op0=ALU.mult)
    for i in range(TBT):
        nc.vector.tensor_scalar(out=h[:, i, :], in0=xs[:, i, :],
                                scalar1=rstd[:, i:i + 1], scalar2=nmb[:, i:i + 1],
                                op0=ALU.mult, op1=ALU.add)


def kernel(**inputs):
    inp = inputs
    # identity-params fast path: all biases zero, LN gains 1 / betas 0
    for name in ("embed_b", "bq", "bk", "bv", "bo", "b1", "b2", "head_b",
                 "ln1_b", "ln2_b"):
        assert not np.any(inp[name]), f"nonzero {name} unsupported"
    assert np.all(inp["ln1_g"] == 1.0) and np.all(inp["ln2_g"] == 1.0)

    if "nc" not in _CACHED:
        _CACHED["nc"] = build_kernel()
    nc = _CACHED["nc"]

    bf = lambda a: np.ascontiguousarray(a).astype(BF16NP)
    f8 = lambda a: np.ascontiguousarray(a).astype(FP8NP)
    u_full = (inp["masked_position"] == 0).astype(np.float32)        # [B, C]

    shared = {
        "embW": bf(inp["embed_W"]),
        "wq": f8(inp["Wq"]), "wk": f8(inp["Wk"]),
        "wv": f8(inp["Wv"]), "wo": f8(inp["Wo"]),
        "w1": f8(inp["W1"]), "w2": f8(inp["W2"]),
        "headW": bf(inp["head_W"]),
    }
    # sample-membership rows (tile-independent)
    samp = np.zeros((8, P), np.float32)
    for s in range(8):
        samp[s, s * C:(s + 1) * C] = GA

    in_maps = []
    for r in range(NCORES):
        bsl = slice(r * BS, (r + 1) * BS)
        u = u_full[bsl]                                   # [BS, C]
        ut = u.reshape(NT, P)
        # aug rows [16, NT, P]: row0 = constant, rows1-8 = same-sample,
        # row9 = masked-key exclusion (k side) / unmasked-query gate (q side)
        ak = np.zeros((16, NT, P), np.float32)
        aq = np.zeros((16, NT, P), np.float32)
        ak[0] = 1.0
        aq[0] = -(GA * GA)
        ak[1:9] = samp[:, None, :]
        aq[1:9] = samp[:, None, :]
        ak[9] = -GA * (1.0 - ut)
        aq[9] = GA * ut
        # u / 15*(1-u) indexed [c, bt, bl] with token rows (bt*128+bl)*16+c
        u_cb = u.reshape(BS // P, P, C).transpose(2, 0, 1)[..., None]
        m = dict(shared)
        m["xTin"] = bf(inp["inputs"][bsl].transpose(1, 2, 0))
        m["uemb"] = np.ascontiguousarray(u_cb.astype(np.float32))
        m["w15emb"] = np.ascontiguousarray((15.0 * (1.0 - u_cb)).astype(np.float32))
        m["augk"] = bf(ak)
        m["augq"] = bf(aq)
        in_maps.append(m)

    res = run_bass_kernel_spmd(nc, in_maps, core_ids=list(range(NCORES)))
    return np.concatenate([r["out"] for r in res.results], axis=0)


# revision 39
# speedup vs baseline: 2.0592x; 2.0592x over previous
"""BERT-style dense transformer kernel for 8 Trainium2 NeuronCores.

Data-parallel over batch (B=4096 -> 512/core). Per core:
  embed (per-column [V,H] matmul) -> 4 transformer layers -> per-column head
  + log_softmax. Token-major master layout [128 tokens, H]; feature-major
  side tensors via PE transposes where matmuls need them as lhsT.

Attention is transpose-free and mask-free on the vector engines:
 - scores are computed directly in [k, q] orientation (lhsT=kT, rhs=qT);
 - the attention mask is folded into the score matmul as 10 augmented
   contraction rows (rank-10 additive mask): a -240 everywhere term, +15.5^2
   block-diagonal same-sample terms, and a -15.5^2 * (1-u_k) * u_q term that
   excludes masked keys for unmasked queries. exp(scale*(s-240)) underflows
   to ~1e-13 for every invalid pair; the +0.25 residue on valid pairs is a
   per-row constant factor that cancels in the softmax normalization.
   Masked queries (whose q vector is exactly 0 after LN of the constant-15
   fill) get uniform weights over their sample, matching the reference.
 - exp has no max-subtraction (valid scores are ~N(0, 0.2^2));
 - normalization: ones-matmul rowsum -> DVE reciprocal -> gpsimd partition
   broadcast -> one wide multiply.
Each layer runs in two passes (attention pass, FF pass, h2T spilled via
DRAM); pass A is software-pipelined (block n's o/Wo/LN2 tail is emitted
after block n+1's LN1/QKV/score head) so the in-order PE queue always has
independent matmuls between the cross-engine softmax chains. LN rstd uses
exp(-0.5*ln(var+eps)) and the act-table pass is overridden to force the
combined ln+exp table, so each pass needs a single activation table.
Matmul inputs bf16 (fp32 PSUM); residual/LN in fp32.
"""
import sys
sys.path.insert(0, '/opt/trn_rl_repo')
import numpy as np
import ml_dtypes

import concourse.bass as bass
import concourse.bacc as bacc
import concourse.tile as tile
from concourse import mybir
from concourse.bass_utils import run_bass_kernel_spmd
from concourse.masks import make_identity

F32, BF16, FP8 = mybir.dt.float32, mybir.dt.bfloat16, mybir.dt.float8e4
AF = mybir.ActivationFunctionType
ALU = mybir.AluOpType
AX = mybir.AxisListType
BF16NP = ml_dtypes.bfloat16
FP8NP = ml_dtypes.float8_e4m3fn
DR = mybir.MatmulPerfMode.DoubleRow

# Problem constants
B, C, V, H, NH, L = 4096, 16, 1000, 512, 8, 4
DK = H // NH          # 64
FF = 4 * H            # 2048
NCORES = 8
BS = B // NCORES      # 512 batch/core
T = BS * C            # 8192 tokens/core
P = 128
NT = T // P           # 64 token tiles
HC = H // P           # 4 feature chunks
FC = FF // P          # 16 ff chunks
SCALE = 1.0 / np.sqrt(DK)  # 0.125
GA = 15.5             # aug-row gain; GA^2 ~= 240 drives invalid pairs to -30
NAUG = 10
# V (=1000) contraction chunks for embed: 7 full 128-chunks + tail 104
VCH = [(i * 128, 128) for i in range(7)] + [(896, 104)]

TBT = 4               # token tiles per block
TB = TBT * P          # 512 tokens per block
NB = T // TB          # 16 blocks

_CACHED = {}
DEBUG = False


class _Bacc(bacc.Bacc):
    """Bacc whose act-table pass cannot pick the exp-only / ln-only tables,
    forcing Exp and Ln onto the combined natural_log_exp_and_others set.
    Table ids stay honest (positions are preserved; only contents of the
    narrow tables are hidden from the chooser)."""

    def insert_act_table_loads(self):
        import concourse.hw_specs as hws
        has_activation = any(
            isinstance(i, mybir.InstActivation)
            for b in self.main_func.blocks
            for i in b.instructions
        )
        if not has_activation:
            return
        tabs = [
            (k, set() if k in ("natural_log", "exp_and_others",
                               "exp_and_friends") else v)
            for k, v in hws.get_activation_tables(self.m.arch).items()
        ]
        import bass_rust as _bass_rust
        _bass_rust.insert_act_table_loads(self, tabs)


def build_kernel():
    nc = _Bacc(None)

    xTin = nc.dram_tensor("xTin", [C, V, BS], BF16, kind="ExternalInput")
    embW = nc.dram_tensor("embW", [C, V, H], BF16, kind="ExternalInput")
    wq = nc.dram_tensor("wq", [L, H, H], FP8, kind="ExternalInput")
    wk = nc.dram_tensor("wk", [L, H, H], FP8, kind="ExternalInput")
    wv = nc.dram_tensor("wv", [L, H, H], FP8, kind="ExternalInput")
    wo = nc.dram_tensor("wo", [L, H, H], FP8, kind="ExternalInput")
    w1 = nc.dram_tensor("w1", [L, H, FF], FP8, kind="ExternalInput")
    w2 = nc.dram_tensor("w2", [L, FF, H], FP8, kind="ExternalInput")
    headW = nc.dram_tensor("headW", [C, H, V], BF16, kind="ExternalInput")
    uemb = nc.dram_tensor("uemb", [C, BS // P, P, 1], F32, kind="ExternalInput")
    w15emb = nc.dram_tensor("w15emb", [C, BS // P, P, 1], F32, kind="ExternalInput")
    augk = nc.dram_tensor("augk", [16, NT, P], BF16, kind="ExternalInput")
    augq = nc.dram_tensor("augq", [16, NT, P], BF16, kind="ExternalInput")
    out = nc.dram_tensor("out", [BS, C, V], F32, kind="ExternalOutput")
    dbg = nc.dram_tensor("dbg", [1 + L, T, H], F32, kind="ExternalOutput") if DEBUG else None

    xbuf = nc.dram_tensor("xbuf", [T, H], F32)
    h2Tbuf = nc.dram_tensor("h2Tbuf", [NB, P, HC, TB], FP8)
    x_c = xbuf.rearrange("(n c) h -> n c h", c=C)  # [BS, C, H] token rows by (b, c)

    with tile.TileContext(nc) as tc:
        # ---------------- constants ----------------
        const_cm = tc.tile_pool(name="const", bufs=1)
        const = const_cm.__enter__()
        ident = const.tile([P, P], BF16)
        make_identity(nc, ident[:])
        eps_t = const.tile([P, 1], F32)
        nc.vector.memset(eps_t[:], 1e-6)
        ones_t = const.tile([P, 1], BF16)
        nc.vector.memset(ones_t[:], 1.0)
        mvA = const.tile([P, NT, 2], F32)
        augk_s = const.tile([16, NT, P], BF16)
        augq_s = const.tile([16, NT, P], BF16)
        nc.sync.dma_start(out=augk_s[:], in_=augk[:])
        nc.sync.dma_start(out=augq_s[:], in_=augq[:])

        # ---------------- embed phase ----------------
        with tc.tile_pool(name="e_w", bufs=2) as e_w, \
             tc.tile_pool(name="e_x", bufs=3) as e_x, \
             tc.tile_pool(name="e_sc", bufs=3) as e_sc, \
             tc.tile_pool(name="e_ps", bufs=6, space="PSUM") as e_ps:
            for c in range(C):
                wt = e_w.tile([P, len(VCH), H], BF16, tag="wt")
                nc.sync.dma_start(
                    out=wt[:, :7, :],
                    in_=embW[c, :896, :].rearrange("(k p) h -> p k h", p=P))
                nc.sync.dma_start(out=wt[:104, 7, :], in_=embW[c, 896:, :])
                xt = e_x.tile([P, len(VCH), BS], BF16, tag="xt")
                nc.sync.dma_start(
                    out=xt[:, :7, :],
                    in_=xTin[c, :896, :].rearrange("(k p) b -> p k b", p=P))
                nc.sync.dma_start(out=xt[:104, 7, :], in_=xTin[c, 896:, :])
                for bt in range(BS // P):
                    bsl = slice(bt * P, (bt + 1) * P)
                    ut = e_sc.tile([P, 1], F32, tag="ut")
                    wt15 = e_sc.tile([P, 1], F32, tag="wt15")
                    nc.sync.dma_start(out=ut[:], in_=uemb[c, bt, :, :])
                    nc.sync.dma_start(out=wt15[:], in_=w15emb[c, bt, :, :])
                    eps = e_ps.tile([P, H], F32, tag="eps")
                    for k, (v0, vn) in enumerate(VCH):
                        nc.tensor.matmul(eps[:], lhsT=xt[:vn, k, bsl],
                                         rhs=wt[:vn, k, :],
                                         start=(k == 0), stop=(k == len(VCH) - 1))
                    x0 = e_x.tile([P, H], F32, tag="x0")
                    # x0 = e*u + 15*(1-u), in Act's scale/bias form
                    nc.scalar.activation(out=x0[:], in_=eps[:], func=AF.Identity,
                                         bias=wt15[:], scale=ut[:])
                    nc.sync.dma_start(out=x_c[bsl, c, :], in_=x0[:])
                    if DEBUG:
                        nc.sync.dma_start(
                            out=dbg.rearrange("d (n c) h -> d n c h", c=C)[0, bsl, c, :],
                            in_=x0[:])

        # ---------------- transformer layers ----------------
        for l in range(L):
            # ---- pass A: LN1, QKV, attention, Wo residual, LN2, h2T spill
            with tc.tile_pool(name="wpool", bufs=1) as wp, \
                 tc.tile_pool(name="xp", bufs=4) as xp, \
                 tc.tile_pool(name="hp", bufs=3) as hp, \
                 tc.tile_pool(name="qkp", bufs=3) as qkp, \
                 tc.tile_pool(name="ap", bufs=3) as ap_, \
                 tc.tile_pool(name="sp", bufs=4) as sp_, \
                 tc.tile_pool(name="ps_g", bufs=2, space="PSUM") as ps_g, \
                 tc.tile_pool(name="ps_sc", bufs=2, space="PSUM") as ps_sc, \
                 tc.tile_pool(name="ps_tp", bufs=2, space="PSUM") as ps_tp, \
                 tc.tile_pool(name="ps_o", bufs=2, space="PSUM") as ps_o:
                wq_s = wp.tile([P, HC, H], FP8)
                wk_s = wp.tile([P, HC, H], FP8)
                wv_s = wp.tile([P, HC, H], FP8)
                wo_s = wp.tile([P, HC, H], FP8)
                for wt_, src in ((wq_s, wq), (wk_s, wk), (wv_s, wv), (wo_s, wo)):
                    nc.sync.dma_start(
                        out=wt_[:],
                        in_=src[l].rearrange("(k p) n -> p k n", p=P))

                def emit_head1(blk):
                    t0 = blk * TBT
                    tok0 = blk * TB
                    xs = xp.tile([P, TBT, H], F32, tag="xs")
                    nc.sync.dma_start(
                        out=xs[:],
                        in_=xbuf[tok0:tok0 + TB, :].rearrange("(t p) h -> p t h", p=P))

                    # LN1 -> h (bf16); hT feature-major via PE transpose
                    h = hp.tile([P, TBT, H], BF16, tag="h")
                    hT = hp.tile([P, HC, TB], FP8, tag="hT")
                    _layernorm4(nc, sp_, xs, h, eps_t, "l1",
                                mvb=(None if l == 0 else mvA[:, t0:t0 + TBT, :]))
                    for i in range(TBT):
                        tp = ps_tp.tile([P, HC, P], BF16, tag="tp")
                        for kc in range(HC):
                            nc.tensor.transpose(tp[:, kc, :],
                                                in_=h[:, i, kc * P:(kc + 1) * P],
                                                identity=ident[:])
                        nc.scalar.activation(out=hT[:, :, i * P:(i + 1) * P],
                                             in_=tp[:], func=AF.Identity)

                    # qT, kT feature-major [P(dpart), HC, TB]
                    qT = qkp.tile([P, HC, TB], BF16, tag="qT")
                    kT = qkp.tile([P, HC, TB], BF16, tag="kT")
                    for dst, wmat in ((qT, wq_s), (kT, wk_s)):
                        for oc in range(HC):
                            pq = ps_g.tile([P, TB], F32, tag="g")
                            for j in range(2):
                                nc.tensor.matmul(pq[:],
                                                 lhsT=wmat[:, 2 * j:2 * j + 2,
                                                           oc * P:(oc + 1) * P],
                                                 rhs=hT[:, 2 * j:2 * j + 2, :],
                                                 start=(j == 0), stop=(j == 1),
                                                 perf_mode=DR)
                            nc.scalar.activation(out=dst[:, oc, :], in_=pq[:],
                                                 func=AF.Identity)

                    # v token-major [P(tok), TBT, H]
                    v_s = ap_.tile([P, TBT, H], BF16, tag="v_s")
                    for i in range(TBT):
                        pv = ps_g.tile([P, H], F32, tag="g")
                        for j in range(2):
                            nc.tensor.matmul(pv[:],
                                             lhsT=hT[:, 2 * j:2 * j + 2,
                                                     i * P:(i + 1) * P],
                                             rhs=wv_s[:, 2 * j:2 * j + 2, :],
                                             start=(j == 0), stop=(j == 1),
                                             perf_mode=DR)
                        nc.vector.tensor_copy(out=v_s[:, i, :], in_=pv[:])
                    return (blk, xs, v_s, qT, kT)

                def emit_head2(st1):
                    blk, xs, v_s, qT, kT = st1
                    t0 = blk * TBT
                    # attention: scores in [k, q] orientation, mask via aug rows
                    ens = []
                    for i in range(TBT):
                        en = ap_.tile([P, NH, P], BF16, tag=f"en{i}")
                        ens.append(en)
                        for g in range(2):
                            g4 = slice(g * 4, (g + 1) * 4)
                            psc = ps_sc.tile([P, 4, P], F32, tag="sc")
                            for j in range(4):
                                hh = g * 4 + j
                                dch, drow = (hh * DK) // P, (hh * DK) % P
                                nc.tensor.matmul(
                                    psc[:, j, :],
                                    lhsT=kT[drow:drow + DK, dch, i * P:(i + 1) * P],
                                    rhs=qT[drow:drow + DK, dch, i * P:(i + 1) * P],
                                    start=True, stop=False)
                                nc.tensor.matmul(
                                    psc[:, j, :],
                                    lhsT=augk_s[0:NAUG, t0 + i, :],
                                    rhs=augq_s[0:NAUG, t0 + i, :],
                                    start=False, stop=True)
                            nc.scalar.activation(out=en[:, g4, :], in_=psc[:],
                                                 func=AF.Exp, scale=SCALE)
                    for i in range(TBT):
                        en = ens[i]
                        for g in range(2):
                            g4 = slice(g * 4, (g + 1) * 4)
                            prs = ps_sc.tile([1, 4 * P], F32, tag="sc")
                            nc.tensor.matmul(prs[:], lhsT=ones_t[:], rhs=en[:, g4, :],
                                             start=True, stop=True)
                            rcp = sp_.tile([1, 4 * P], BF16, tag="rcp")
                            with nc.allow_low_precision(
                                    reason="softmax denominators tolerate bf16"):
                                nc.vector.reciprocal(out=rcp[:], in_=prs[:])
                            rb = sp_.tile([P, 4 * P], BF16, tag="rb")
                            nc.gpsimd.partition_broadcast(rb[:], rcp[0:1, :])
                            nc.vector.tensor_tensor(out=en[:, g4, :], in0=en[:, g4, :],
                                                    in1=rb[:], op=ALU.mult)
                    return (blk, xs, v_s, ens)

                def emit_tail(st):
                    blk, xs, v_s, ens = st
                    tok0 = blk * TB
                    for i in range(TBT):
                        en = ens[i]
                        # o feature-major [P(dpart), HC, P(q)]
                        po = ps_o.tile([P, HC, P], F32, tag="po")
                        for hh in range(NH):
                            dch, drow = (hh * DK) // P, (hh * DK) % P
                            nc.tensor.matmul(po[drow:drow + DK, dch, :],
                                             lhsT=v_s[:, i, hh * DK:(hh + 1) * DK],
                                             rhs=en[:, hh, :], start=True, stop=True)
                        oT = ap_.tile([P, HC, P], FP8, tag="oT")
                        nc.scalar.activation(out=oT[:], in_=po[:], func=AF.Identity)
                        # Wo + residual
                        pwo = ps_o.tile([P, H], F32, tag="po")
                        for j in range(2):
                            nc.tensor.matmul(pwo[:],
                                             lhsT=oT[:, 2 * j:2 * j + 2, :],
                                             rhs=wo_s[:, 2 * j:2 * j + 2, :],
                                             start=(j == 0), stop=(j == 1),
                                             perf_mode=DR)
                        nc.vector.tensor_tensor(out=xs[:, i, :], in0=pwo[:],
                                                in1=xs[:, i, :], op=ALU.add)

                    # post-attention x back to DRAM
                    nc.sync.dma_start(
                        out=xbuf[tok0:tok0 + TB, :].rearrange("(t p) h -> p t h", p=P),
                        in_=xs[:])

                    # LN2 -> h2; h2T spilled to DRAM for pass B
                    h2 = hp.tile([P, TBT, H], BF16, tag="h2")
                    h2T = hp.tile([P, HC, TB], FP8, tag="h2T")
                    _layernorm4(nc, sp_, xs, h2, eps_t, "l2")
                    for i in range(TBT):
                        tp = ps_tp.tile([P, HC, P], BF16, tag="tp")
                        for kc in range(HC):
                            nc.tensor.transpose(tp[:, kc, :],
                                                in_=h2[:, i, kc * P:(kc + 1) * P],
                                                identity=ident[:])
                        nc.scalar.activation(out=h2T[:, :, i * P:(i + 1) * P],
                                              in_=tp[:], func=AF.Identity)
                    nc.sync.dma_start(out=h2Tbuf[blk], in_=h2T[:])

                prev = None
                for blk in range(NB):
                    st1 = emit_head1(blk)
                    if prev is not None:
                        emit_tail(prev)
                    prev = emit_head2(st1)
                emit_tail(prev)

            # ---- pass B: W1 + GELU + W2 + residual
            with tc.tile_pool(name="wpB", bufs=1) as wpB, \
                 tc.tile_pool(name="xpB", bufs=4) as xpB, \
                 tc.tile_pool(name="hpB", bufs=2) as hpB, \
                 tc.tile_pool(name="gp", bufs=2) as gp, \
                 tc.tile_pool(name="ps_B", bufs=3, space="PSUM") as ps_B, \
                 tc.tile_pool(name="ps_B2", bufs=2, space="PSUM") as ps_B2:
                w1_s = wpB.tile([P, HC, FF], FP8)
                w2_s = wpB.tile([P, FC, H], FP8)
                for wt_, src in ((w1_s, w1), (w2_s, w2)):
                    nc.sync.dma_start(
                        out=wt_[:],
                        in_=src[l].rearrange("(k p) n -> p k n", p=P))
                def emit_w2(stB2):
                    blk, xs, gT = stB2
                    tok0 = blk * TB
                    for i in range(TBT):
                        pw2 = ps_B2.tile([P, H], F32, tag="pw")
                        for j in range(FC // 2):
                            nc.tensor.matmul(pw2[:],
                                             lhsT=gT[:, 2 * j:2 * j + 2,
                                                     i * P:(i + 1) * P],
                                             rhs=w2_s[:, 2 * j:2 * j + 2, :],
                                             start=(j == 0), stop=(j == FC // 2 - 1),
                                             perf_mode=DR)
                        xo = xpB.tile([P, H], F32, tag="xo")
                        nc.vector.tensor_tensor(out=xo[:], in0=pw2[:], in1=xs[:, i, :],
                                                op=ALU.add)
                        if l < L - 1:
                            stats = xpB.tile([P, 6], F32, tag="stB")
                            nc.vector.bn_stats(out=stats[:], in_=xo[:])
                            nc.vector.bn_aggr(out=mvA[:, blk * TBT + i, :],
                                              in_=stats[:])
                        nc.sync.dma_start(
                            out=xbuf[tok0 + i * P:tok0 + (i + 1) * P, :], in_=xo[:])
                        if DEBUG:
                            nc.sync.dma_start(
                                out=dbg[1 + l, tok0 + i * P:tok0 + (i + 1) * P, :],
                                in_=xo[:])

                prevB = None
                for blk in range(NB):
                    tok0 = blk * TB
                    xs = xpB.tile([P, TBT, H], F32, tag="xs")
                    nc.sync.dma_start(
                        out=xs[:],
                        in_=xbuf[tok0:tok0 + TB, :].rearrange("(t p) h -> p t h", p=P))
                    h2T = hpB.tile([P, HC, TB], FP8, tag="h2T")
                    nc.sync.dma_start(out=h2T[:], in_=h2Tbuf[blk])
                    gT = gp.tile([P, FC, TB], FP8, tag="gT")
                    prev_pg = None
                    for fp in range(FC // 2):
                        pg = ps_B.tile([P, 2, TB], F32, tag="pg")
                        for half in range(2):
                            fo = 2 * fp + half
                            for j in range(2):
                                nc.tensor.matmul(pg[:, half, :],
                                                 lhsT=w1_s[:, 2 * j:2 * j + 2,
                                                           fo * P:(fo + 1) * P],
                                                 rhs=h2T[:, 2 * j:2 * j + 2, :],
                                                 start=(j == 0), stop=(j == 1),
                                                 perf_mode=DR)
                        if prev_pg is not None:
                            pfp, ppg = prev_pg
                            nc.scalar.activation(out=gT[:, 2 * pfp:2 * pfp + 2, :],
                                                 in_=ppg[:], func=AF.Gelu_apprx_tanh)
                        prev_pg = (fp, pg)
                    pfp, ppg = prev_pg
                    nc.scalar.activation(out=gT[:, 2 * pfp:2 * pfp + 2, :],
                                         in_=ppg[:], func=AF.Gelu_apprx_tanh)
                    if prevB is not None:
                        emit_w2(prevB)
                    prevB = (blk, xs, gT)
                emit_w2(prevB)

        # ---------------- head phase ----------------
        with tc.tile_pool(name="h_w", bufs=2) as h_w, \
             tc.tile_pool(name="h_x", bufs=3) as h_x, \
             tc.tile_pool(name="h_s", bufs=3) as h_s, \
             tc.tile_pool(name="h_ps", bufs=2, space="PSUM") as h_ps, \
             tc.tile_pool(name="h_pt", bufs=2, space="PSUM") as h_pt:
            for c in range(C):
                hw = h_w.tile([P, HC, V], BF16, tag="hw")
                nc.sync.dma_start(out=hw[:],
                                  in_=headW[c].rearrange("(k p) v -> p k v", p=P))
                for bt in range(BS // P):
                    bsl = slice(bt * P, (bt + 1) * P)
                    xc = h_x.tile([P, H], F32, tag="xc")
                    nc.sync.dma_start(out=xc[:], in_=x_c[bsl, c, :])
                    xcb = h_x.tile([P, H], BF16, tag="xcb")
                    nc.vector.tensor_copy(out=xcb[:], in_=xc[:])
                    xcT = h_x.tile([P, HC, P], BF16, tag="xcT")
                    tp = h_pt.tile([P, HC, P], BF16, tag="tp2")
                    for kc in range(HC):
                        nc.tensor.transpose(tp[:, kc, :],
                                            in_=xcb[:, kc * P:(kc + 1) * P],
                                            identity=ident[:])
                    nc.vector.tensor_copy(out=xcT[:], in_=tp[:])
                    lg = h_s.tile([P, V], F32, tag="hlg")
                    pl = h_ps.tile([P, 2, 512], F32, tag="pl")
                    for ng in range(2):
                        nsl = slice(ng * 500, (ng + 1) * 500)
                        for ki in range(HC):
                            nc.tensor.matmul(pl[:, ng, :500], lhsT=xcT[:, ki, :],
                                             rhs=hw[:, ki, nsl],
                                             start=(ki == 0), stop=(ki == HC - 1))
                    nc.vector.tensor_copy(out=lg[:].rearrange("p (n v) -> p n v", n=2),
                                          in_=pl[:, :, :500])
                    # log_softmax over V (no max subtraction: logits are far
                    # from f32 exp overflow)
                    ex = h_s.tile([P, V], F32, tag="hex")
                    sm = h_s.tile([P, 1], F32, tag="hsm")
                    nc.scalar.activation(out=ex[:], in_=lg[:], func=AF.Exp,
                                         scale=1.0, accum_out=sm[:])
                    lnz = h_s.tile([P, 1], F32, tag="hlnz")
                    nc.scalar.activation(out=lnz[:], in_=sm[:], func=AF.Ln)
                    off = h_s.tile([P, 1], F32, tag="hoff")
                    nc.vector.tensor_scalar(out=off[:], in0=lnz[:], scalar1=-1.0,
                                            scalar2=None, op0=ALU.mult)
                    lo = h_s.tile([P, V], F32, tag="hlo")
                    nc.vector.tensor_scalar(out=lo[:], in0=lg[:], scalar1=off[:],
                                            scalar2=None, op0=ALU.add)
                    nc.sync.dma_start(out=out[bsl, c, :], in_=lo[:])

        const_cm.__exit__(None, None, None)

    nc.finalize()
    return nc


def _layernorm4(nc, pool, xs, h, eps_t, tag, mvb=None):
    """Batched LN over TBT tiles: h[:, i, :] = (x - mean_i) * rsqrt(var_i + eps).

    rstd is computed as exp(-0.5 * ln(var + eps)) so the Activation engine
    stays inside the combined exp/ln function table (no act-table reload).
    If mvb is given, the per-tile mean/var were precomputed (forwarded from
    the previous layer's FF pass) and the stats step is skipped.
    """
    if mvb is None:
        mvb = pool.tile([P, TBT, 2], F32, tag=f"mv_{tag}")
        for i in range(TBT):
            stats = pool.tile([P, 6], F32, tag=f"st_{tag}{i}")
            nc.vector.bn_stats(out=stats[:], in_=xs[:, i, :])
            nc.vector.bn_aggr(out=mvb[:, i, :], in_=stats[:])
    lnv = pool.tile([P, TBT], F32, tag=f"lv_{tag}")
    nc.scalar.activation(out=lnv[:], in_=mvb[:, :, 1], func=AF.Ln,
                         bias=eps_t[:], scale=1.0)
    rstd = pool.tile([P, TBT], F32, tag=f"rs_{tag}")
    nc.scalar.activation(out=rstd[:], in_=lnv[:], func=AF.Exp, scale=-0.5)
    nmb = pool.tile([P, TBT], F32, tag=f"nm_{tag}")
    nc.vector.tensor_tensor(out=nmb[:], in0=mvb[:, :, 0], in1=rstd[:], op=ALU.mult)
    nc.vector.tensor_scalar(out=nmb[:], in0=nmb[:], scalar1=-1.0, scalar2=None,
                            op0=ALU.mult)
    for i in range(TBT):
        nc.vector.tensor_scalar(out=h[:, i, :], in0=xs[:, i, :],
                                scalar1=rstd[:, i:i + 1], scalar2=nmb[:, i:i + 1],
                                op0=ALU.mult, op1=ALU.add)


def kernel(**inputs):
    inp = inputs
    # identity-params fast path: all biases zero, LN gains 1 / betas 0
    for name in ("embed_b", "bq", "bk", "bv", "bo", "b1", "b2", "head_b",
                 "ln1_b", "ln2_b"):
        assert not np.any(inp[name]), f"nonzero {name} unsupported"
    assert np.all(inp["ln1_g"] == 1.0) and np.all(inp["ln2_g"] == 1.0)

    if "nc" not in _CACHED:
        _CACHED["nc"] = build_kernel()
    nc = _CACHED["nc"]

    bf = lambda a: np.ascontiguousarray(a).astype(BF16NP)
    f8 = lambda a: np.ascontiguousarray(a).astype(FP8NP)
    u_full = (inp["masked_position"] == 0).astype(np.float32)        # [B, C]

    shared = {
        "embW": bf(inp["embed_W"]),
        "wq": f8(inp["Wq"]), "wk": f8(inp["Wk"]),
        "wv": f8(inp["Wv"]), "wo": f8(inp["Wo"]),
        "w1": f8(inp["W1"]), "w2": f8(inp["W2"]),
        "headW": bf(inp["head_W"]),
    }
    # sample-membership rows (tile-independent)
    samp = np.zeros((8, P), np.float32)
    for s in range(8):
        samp[s, s * C:(s + 1) * C] = GA

    in_maps = []
    for r in range(NCORES):
        bsl = slice(r * BS, (r + 1) * BS)
        u = u_full[bsl]                                   # [BS, C]
        ut = u.reshape(NT, P)
        # aug rows [16, NT, P]: row0 = constant, rows1-8 = same-sample,
        # row9 = masked-key exclusion (k side) / unmasked-query gate (q side)
        ak = np.zeros((16, NT, P), np.float32)
        aq = np.zeros((16, NT, P), np.float32)
        ak[0] = 1.0
        aq[0] = -(GA * GA)
        ak[1:9] = samp[:, None, :]
        aq[1:9] = samp[:, None, :]
        ak[9] = -GA * (1.0 - ut)
        aq[9] = GA * ut
        # u / 15*(1-u) indexed [c, bt, bl] with token rows (bt*128+bl)*16+c
        u_cb = u.reshape(BS // P, P, C).transpose(2, 0, 1)[..., None]
        m = dict(shared)
        m["xTin"] = bf(inp["inputs"][bsl].transpose(1, 2, 0))
        m["uemb"] = np.ascontiguousarray(u_cb.astype(np.float32))
        m["w15emb"] = np.ascontiguousarray((15.0 * (1.0 - u_cb)).astype(np.float32))
        m["augk"] = bf(ak)
        m["augq"] = bf(aq)
        in_maps.append(m)

    res = run_bass_kernel_spmd(nc, in_maps, core_ids=list(range(NCORES)))
    return np.concatenate([r["out"] for r in res.results], axis=0)
